# revision 20
# baseline (speedup 1.0000x reference)
"""Trainium2 Bass kernel for nn_BidirectionalRNN (3-layer LN-tanh RNN, bidir).

Sharding: 8 cores = 4 batch-shards x 2 directions (l2r on cores 0-3, r2l on
cores 4-7), B_loc=32 batches per core. All cores run the same SPMD program;
per-core inputs differ (direction weights + batch slice).

v3 design (vs v1 baseline at 2.45ms, v2 at 2.71ms):
- Per-unit PSUM tiles (no joint tiles): tile-granular dependency tracking in
  the Tile framework had turned v2's shared per-tau PSUM tiles into hard
  3-unit barriers at every pipeline stage.
- Row-form LN stats: constant [P,1] ones/H stationary, st moving ->
  pst [1,2B] = [mean | meansq]; stats + broadcast share one per-unit PSUM
  combo tile [P, 4B] (stats row at cols 0:2B, bcast block at 2B:4B).
- Newton rsqrt = 7 DVE ops: m2, ve, seed shift, seed sub, y0^2, v*y0^2, and
  a fused (q*-0.5+1.5)*y0 via the AFFINE_MUL_REDUCE custom DVE op.
- mean -> f16 rides ACT straight from PSUM stats (off the DVE chain).
- y = (s - mean_bc) * rstd_bc reads the bcast PSUM block directly (no ac
  copy), or via ACT copy + DVE/GPSIMD (KERNEL_Y env: psum | sbuf | gpsimd).
- Embedding+xproj0 fused on host (Wfused = Wemb_aug @ Wx0, bias0 in the
  ones-row); pre-phase slabs interleave with the recurrence.
- FC moved to the host: h2 tiles stream out over the idle DMA engines; the
  45-col FC matmul + pad gather run on numpy in combine_outputs.
"""

import numpy as np
import ml_dtypes

import concourse.bass as bass
import concourse.bacc as bacc
import concourse.tile as tile
from concourse import mybir
from concourse.bass_utils import run_bass_kernel_spmd

import os
USE_FP16 = os.environ.get("KERNEL_DT16", "f16") == "f16"
NP16 = np.float16 if USE_FP16 else ml_dtypes.bfloat16
Y_ENGINE = os.environ.get("KERNEL_Y", "psum")  # psum | sbuf | gpsimd
USE_AMR = os.environ.get("KERNEL_AMR", "1") == "1"
SQ_DVE = os.environ.get("KERNEL_SQ", "act") == "dve"

H = 512
IN_DIM = 300
NCLS = 45
P = 128
KC = H // P  # 4 chunks
N_CORES = 8
QK = 0x5F3759DF

f32 = mybir.dt.float32
i32 = mybir.dt.int32
f16 = mybir.dt.float16 if USE_FP16 else mybir.dt.bfloat16
Sq = mybir.ActivationFunctionType.Square
Tanh = mybir.ActivationFunctionType.Tanh
add_ = mybir.AluOpType.add
sub_ = mybir.AluOpType.subtract
mul_ = mybir.AluOpType.mult
shr_ = mybir.AluOpType.arith_shift_right


def _view0(ap, reps, width):
    """[P, width] AP -> [P, reps, width] AP re-reading the same cols."""
    return bass.AP(tensor=ap.tensor, offset=ap.offset,
                   ap=[ap.ap[0], [0, reps], [1, width]])


def _view0_mid(ap, width):
    """[P, KC] AP -> [P, KC, width] AP, broadcasting each col along width."""
    return bass.AP(tensor=ap.tensor, offset=ap.offset,
                   ap=[ap.ap[0], ap.ap[1], [0, width]])


def build_nc(T=256, B=32):
    COLS = T * B
    S = 32 * B               # pre-phase slab = 32 timesteps of columns
    MMN = 256                # pre-phase matmul moving width
    NG = S // MMN            # matmul groups per slab (per m chunk)
    n_slabs = COLS // S      # 8
    SPT = T // n_slabs       # steps per slab (32)

    nc = bacc.Bacc(None, target_bir_lowering=False)

    xt_d = nc.dram_tensor("xt", [3, P, COLS], f16, kind="ExternalInput")
    wfused_d = nc.dram_tensor("wfused", [P, 3, H], f16, kind="ExternalInput")
    # recurrence weights: Wh0, Wx1, Wh1, Wx2, Wh2
    wrec_d = nc.dram_tensor("wrec", [5, P, KC, H], f16, kind="ExternalInput")
    bias12_d = nc.dram_tensor("bias12", [P, 2, KC], f32, kind="ExternalInput")
    h2_d = nc.dram_tensor("h2", [P, KC, COLS], f16, kind="ExternalOutput")

    with tile.TileContext(nc) as tc:
        import contextlib
        with contextlib.ExitStack() as ctx:
            const = ctx.enter_context(tc.tile_pool(name="const", bufs=1))
            xtp = ctx.enter_context(tc.tile_pool(name="xtp", bufs=2))
            xpp = ctx.enter_context(tc.tile_pool(name="xpp", bufs=3))
            stp = ctx.enter_context(tc.tile_pool(name="stp", bufs=3))
            hp = ctx.enter_context(tc.tile_pool(name="hp", bufs=2))
            yp = ctx.enter_context(tc.tile_pool(name="yp", bufs=3))
            rcp = ctx.enter_context(tc.tile_pool(name="rcp", bufs=3))
            acp = ctx.enter_context(tc.tile_pool(name="acp", bufs=3))
            nwt = ctx.enter_context(tc.tile_pool(name="nwt", bufs=3))
            ps_bp = ctx.enter_context(tc.tile_pool(name="ps_bp", bufs=2, space="PSUM"))
            ps_u = [ctx.enter_context(tc.tile_pool(name=f"ps_u{l}", bufs=1, space="PSUM"))
                    for l in range(3)]
            ps_c = [ctx.enter_context(tc.tile_pool(name=f"ps_c{l}", bufs=1, space="PSUM"))
                    for l in range(3)]

            wfused_sb = const.tile([P, 3, H], f16)
            nc.sync.dma_start(out=wfused_sb, in_=wfused_d.ap())
            wrec_sb = const.tile([P, 5, KC, H], f16)
            nc.sync.dma_start(out=wrec_sb, in_=wrec_d.ap().rearrange("n p k m -> p n k m"))
            bias12_sb = const.tile([P, 2, KC], f32)
            nc.sync.dma_start(out=bias12_sb, in_=bias12_d.ap())

            ones16 = const.tile([1, P], f16)
            nc.vector.memset(ones16, 1.0)
            sc_ones = const.tile([P, 1], f16)
            nc.vector.memset(sc_ones, 1.0 / H)
            qkrow = const.tile([1, B], i32)
            nc.vector.memset(qkrow, QK)

            wh_idx = [0, 2, 4]   # Wh0, Wh1, Wh2 in wrec
            wx_idx = [None, 1, 3]

            # ---- slab pre-phase: xproj0 (+bias0) for SPT steps ----
            def emit_slab_start(sl):
                xt_t = xtp.tile([P, 3, S], f16, tag="xt", name="xt_t")
                c0 = sl * S
                for k in range(3):
                    nc.sync.dma_start(out=xt_t[:, k, :],
                                      in_=xt_d.ap()[k, :, c0:c0 + S])
                xp_t = xpp.tile([P, SPT, KC, B], f16, tag="xp", name="xp_t")
                return (xt_t, xp_t)

            def emit_slab_chunk(state, m, ns):
                xt_t, xp_t = state
                psx = ps_bp.tile([P, MMN], f32, tag="bp", name="psx")
                for k in range(3):
                    nc.tensor.matmul(psx, wfused_sb[:, k, bass.ts(m, P)],
                                     xt_t[:, k, bass.ts(ns, MMN)],
                                     start=(k == 0), stop=(k == 2))
                tpc = MMN // B  # steps covered by one chunk
                nc.scalar.copy(xp_t[:, ns * tpc:(ns + 1) * tpc, m, :], psx)

            slab_states = {}
            for sl in range(min(2, n_slabs)):
                slab_states[sl] = emit_slab_start(sl)
                for m in range(KC):
                    for ns in range(NG):
                        emit_slab_chunk(slab_states[sl], m, ns)

            # ---- recurrence state ----
            h = {}
            for l in range(3):
                h0 = hp.tile([P, KC, B], f16, tag=f"h{l}", name=f"h{l}")
                nc.vector.memset(h0, 0.0)
                h[l] = h0

            def emit_pre_part(t, l, ps, ms):
                # complete 8-matmul (4 for l0) accumulation group per m-chunk
                for m in ms:
                    n_mm = KC * (2 if l > 0 else 1)
                    i = 0
                    for k in range(KC):
                        nc.tensor.matmul(ps[:, m, :],
                                         wrec_sb[:, wh_idx[l], k, bass.ts(m, P)],
                                         h[l][:, k, :],
                                         start=(i == 0), stop=(i == n_mm - 1))
                        i += 1
                    if l > 0:
                        for k in range(KC):
                            nc.tensor.matmul(ps[:, m, :],
                                             wrec_sb[:, wx_idx[l], k, bass.ts(m, P)],
                                             h[l - 1][:, k, :],
                                             start=False, stop=(i == n_mm - 1))
                            i += 1
                return ps

            def emit_st(t, l, ps):
                st = stp.tile([P, KC, 2 * B], f16, tag="st", name="st")
                if l == 0:
                    xp_t = slab_states[t // SPT][1]
                    nc.vector.tensor_tensor(st[:, :, :B], ps,
                                            xp_t[:, t % SPT, :, :], add_)
                else:
                    nc.vector.tensor_tensor(
                        st[:, :, :B], ps,
                        _view0_mid(bias12_sb[:, l - 1, :], B), add_)
                if SQ_DVE:
                    nc.vector.tensor_tensor(st[:, :, B:], st[:, :, :B],
                                            st[:, :, :B], mul_)
                else:
                    nc.scalar.activation(st[:, :, B:], st[:, :, :B], Sq)
                return st

            def emit_stats(t, l, st):
                combo = ps_c[l].tile([P, 4 * B], f32, tag="c", name=f"combo{l}")
                pst = combo[0:1, 0:2 * B]
                for k in range(KC):
                    nc.tensor.matmul(pst, sc_ones, st[:, k, :],
                                     start=(k == 0), stop=(k == KC - 1))
                return combo

            def emit_m2mc(t, l, combo):
                pst = combo[0:1, 0:2 * B]
                rc = rcp.tile([1, 2 * B], f16, tag="rc", name="rc")
                m2 = nwt.tile([1, B], f32, tag="m2", name="m2")
                # mean^2 on ACT straight from PSUM; mean f16 copy off-chain
                nc.scalar.activation(m2, pst[:, :B], Sq)
                nc.scalar.copy(rc[:, B:], pst[:, :B])
                return rc, m2

            def emit_newton(t, l, combo, rcm2=None):
                pst = combo[0:1, 0:2 * B]
                rc, m2 = rcm2 if rcm2 else emit_m2mc(t, l, combo)
                ve = nwt.tile([1, B], f32, tag="ve", name="ve")
                nc.vector.tensor_tensor(ve, pst[:, B:], m2, sub_)
                ui = nwt.tile([1, B], i32, tag="ui", name="ui")
                nc.vector.tensor_scalar(ui, ve.bitcast(i32), 1, None, shr_)
                y0i = nwt.tile([1, B], i32, tag="y0i", name="y0i")
                nc.vector.tensor_tensor(y0i, qkrow, ui, sub_)
                y0 = y0i.bitcast(f32)
                y2 = nwt.tile([1, B], f32, tag="y2", name="y2")
                nc.vector.tensor_tensor(y2, y0, y0, mul_)
                q = nwt.tile([1, B], f32, tag="q", name="q")
                nc.vector.tensor_tensor(q, ve, y2, mul_)
                if USE_AMR:
                    acc = nwt.tile([1, 1], f32, tag="acc", name="acc")
                    # rc[:, :B] = (q * -0.5 + 1.5) * y0  (fused Newton step)
                    nc.vector.affine_mul_reduce(rc[:, :B], acc, q, y0, -0.5, 1.5)
                else:
                    e = nwt.tile([1, B], f32, tag="e", name="e")
                    nc.vector.tensor_scalar(e, q, -0.5, 1.5, mul_, add_)
                    nc.vector.tensor_tensor(rc[:, :B], y0, e, mul_)
                return rc

            def emit_bcast(t, l, rc, combo):
                bcp = combo[:, 2 * B:4 * B]
                nc.tensor.matmul(bcp, ones16, rc, start=True, stop=True)
                return bcp

            def emit_y_tanh(t, l, st, bcp):
                s_ = st[:, :, :B]
                if Y_ENGINE == "psum":
                    ysub = yp.tile([P, KC, B], f16, tag="ysub", name="ysub")
                    nc.vector.tensor_tensor(ysub, s_,
                                            _view0(bcp[:, B:], KC, B), sub_)
                    ymul = yp.tile([P, KC, B], f16, tag="ymul", name="ymul")
                    nc.vector.tensor_tensor(ymul, ysub,
                                            _view0(bcp[:, :B], KC, B), mul_)
                else:
                    ac = acp.tile([P, 2 * B], f16, tag="ac", name="ac")
                    nc.scalar.copy(ac, bcp)
                    eng = nc.gpsimd if Y_ENGINE == "gpsimd" else nc.vector
                    ysub = yp.tile([P, KC, B], f16, tag="ysub", name="ysub")
                    eng.tensor_tensor(ysub, s_, _view0(ac[:, B:], KC, B), sub_)
                    ymul = yp.tile([P, KC, B], f16, tag="ymul", name="ymul")
                    eng.tensor_tensor(ymul, ysub, _view0(ac[:, :B], KC, B), mul_)
                hn = hp.tile([P, KC, B], f16, tag=f"h{l}", name=f"h{l}n")
                nc.scalar.activation(hn, ymul, Tanh)
                h[l] = hn
                if l == 2:
                    nc.sync.dma_start(out=h2_d.ap()[:, :, t * B:(t + 1) * B],
                                      in_=hn)

            # slab pre-work for slab sl spread over taus early in slab sl-2
            slab_sched = {}
            for sl in range(2, n_slabs):
                base = (sl - 2) * SPT + 2
                slab_sched.setdefault(base - 1, []).append(("start", sl))
                for g in range(KC * NG):
                    slab_sched.setdefault(base + 2 * g, []).append(
                        (sl, g // NG, g % NG))

            def emit_tail(u):
                # bcast + y + tanh (+ h2 DMA) for one unit
                bc = emit_bcast(u["t"], u["l"], u["rc"], u["combo"])
                emit_y_tanh(u["t"], u["l"], u["st"], bc)

            prev1 = prev2 = None
            for tau in range(T + 3):
                def mk(t, l):
                    return {"t": t, "l": l} if 0 <= t < T else None
                u0, u1, u2 = mk(tau, 0), mk(tau - 1, 1), mk(tau - 2, 2)
                for act in slab_sched.get(tau, []):
                    if act[0] == "start":
                        slab_states[act[1]] = emit_slab_start(act[1])
                # A: leftover l1 tail from previous tau
                if prev1:
                    emit_tail(prev1)
                # D: l0 front
                if u0:
                    u0["ps"] = ps_u[0].tile([P, KC, B], f32, tag="pre",
                                            name="pre0")
                    emit_pre_part(u0["t"], 0, u0["ps"], range(KC))
                    u0["st"] = emit_st(u0["t"], 0, u0["ps"])
                # C: leftover l2 tail
                if prev2:
                    emit_tail(prev2)
                # E: l1 pre interleaved with l0 stats+newton
                if u1:
                    u1["ps"] = ps_u[1].tile([P, KC, B], f32, tag="pre",
                                            name="pre1")
                    emit_pre_part(u1["t"], 1, u1["ps"], (0, 1))
                if u0:
                    u0["combo"] = emit_stats(u0["t"], 0, u0["st"])
                    u0["rc"] = emit_newton(u0["t"], 0, u0["combo"])
                if u1:
                    emit_pre_part(u1["t"], 1, u1["ps"], (2, 3))
                    u1["st"] = emit_st(u1["t"], 1, u1["ps"])
                # F: l2 pre interleaved with l1 stats
                if u2:
                    u2["ps"] = ps_u[2].tile([P, KC, B], f32, tag="pre",
                                            name="pre2")
                    emit_pre_part(u2["t"], 2, u2["ps"], (0, 1))
                if u1:
                    u1["combo"] = emit_stats(u1["t"], 1, u1["st"])
                    u1["rcm2"] = emit_m2mc(u1["t"], 1, u1["combo"])
                if u2:
                    emit_pre_part(u2["t"], 2, u2["ps"], (2, 3))
                # G: slab filler, l0 tail, l1/l2 newtons
                for act in slab_sched.get(tau, []):
                    if act[0] != "start":
                        sl, m, ns = act
                        emit_slab_chunk(slab_states[sl], m, ns)
                if u0:
                    emit_tail(u0)
                if u2:
                    u2["st"] = emit_st(u2["t"], 2, u2["ps"])
                if u1:
                    u1["rc"] = emit_newton(u1["t"], 1, u1["combo"],
                                           u1["rcm2"])
                if u2:
                    u2["combo"] = emit_stats(u2["t"], 2, u2["st"])
                    u2["rc"] = emit_newton(u2["t"], 2, u2["combo"])
                prev1, prev2 = u1, u2

    nc.compile()
    return nc


# ---------------- host-side prep ----------------

def _lay_w(w):
    """[H, M] fp32 -> [P, KC, M] f16 chunk layout."""
    Hh, M = w.shape
    kc = Hh // P
    return np.ascontiguousarray(
        w.reshape(kc, P, M).transpose(1, 0, 2)).astype(NP16)


def make_in_maps(inputs, T=256, B=32):
    """Build the 8 per-core input dicts from the full problem inputs."""
    x = np.asarray(inputs["x"], np.float32)[:, :T]
    rx = np.asarray(inputs["reverse_x"], np.float32)[:, :T]
    W_emb = np.asarray(inputs["W_emb"], np.float32)
    b_emb = np.asarray(inputs["b_emb"], np.float32)

    wemb_aug = np.zeros((3 * P, H), np.float32)
    wemb_aug[:IN_DIM] = W_emb
    wemb_aug[IN_DIM] = b_emb

    dirs = {}
    for d, (xx, sfx) in enumerate([(x, "l2r"), (rx, "r2l")]):
        Wx = np.asarray(inputs[f"Wx_{sfx}"], np.float32)
        bx = np.asarray(inputs[f"bx_{sfx}"], np.float32)
        Wh = np.asarray(inputs[f"Wh_{sfx}"], np.float32)
        bh = np.asarray(inputs[f"bh_{sfx}"], np.float32)
        wrec = np.stack([_lay_w(Wh[0]), _lay_w(Wx[1]), _lay_w(Wh[1]),
                         _lay_w(Wx[2]), _lay_w(Wh[2])])  # [5, P, KC, H]
        wfused = wemb_aug @ Wx[0]
        wfused[IN_DIM] += bx[0] + bh[0]   # bias0 rides the ones-row
        bias12 = np.stack([(bx[1] + bh[1]).reshape(KC, P).T,
                           (bx[2] + bh[2]).reshape(KC, P).T], 1).astype(np.float32)
        dirs[d] = dict(
            x=xx,
            wfused=_lay_w(wfused),           # [P, 3, H]
            wrec=np.ascontiguousarray(wrec),
            bias12=np.ascontiguousarray(bias12),
        )

    n_shard = N_CORES // 2
    in_maps = []
    for core in range(N_CORES):
        d = 0 if core < n_shard else 1
        s = core % n_shard
        dd = dirs[d]
        xc = dd["x"][s * B:(s + 1) * B]  # [B, T, IN]
        xa = np.zeros((3 * P, T * B), np.float32)
        xa[:IN_DIM] = xc.transpose(2, 1, 0).reshape(IN_DIM, T * B)
        xa[IN_DIM] = 1.0
        in_maps.append({
            "xt": np.ascontiguousarray(xa.reshape(3, P, T * B)).astype(NP16),
            "wfused": dd["wfused"],
            "wrec": dd["wrec"],
            "bias12": dd["bias12"],
        })
    return in_maps


def combine_outputs(results, pad_start_index, W_fc, b_fc, T=256, B=32):
    """results: list of 8 dicts with 'h2' [P, KC, T*B]. Returns [128*T, NCLS]."""
    n_shard = N_CORES // 2
    Bfull = n_shard * B
    W_fc = np.asarray(W_fc, np.float32)
    b_fc = np.asarray(b_fc, np.float32)
    L = np.zeros((2, Bfull, T, NCLS), np.float32)
    for core in range(N_CORES):
        d = 0 if core < n_shard else 1
        s = core % n_shard
        h2 = np.asarray(results[core]["h2"], np.float32)  # [P, KC, T*B]
        hfull = h2.transpose(1, 0, 2).reshape(H, T * B)   # h-dim = k*128+p
        wfc_half = W_fc[:H] if d == 0 else W_fc[H:]
        o = wfc_half.T @ hfull                            # [NCLS, T*B]
        o = o.reshape(NCLS, T, B)
        L[d, s * B:(s + 1) * B] = o.transpose(2, 1, 0)
    p = np.asarray(pad_start_index).astype(np.int64)[:, None]
    j = np.arange(T)[None, :]
    idx = np.where(j < p, p - j - 1, j)  # [Bfull, T]
    L2g = np.take_along_axis(L[1], idx[:, :, None], axis=1)
    logits = L[0] + L2g + b_fc
    return logits.reshape(Bfull * T, NCLS)


_NC_CACHE = {}


def kernel(**inputs) -> np.ndarray:
    T = int(inputs["max_length"])
    assert T == 256, f"kernel compiled for T=256, got {T}"
    B = 32
    ln_g = np.asarray(inputs["ln_g"], np.float32)
    ln_b = np.asarray(inputs["ln_b"], np.float32)
    assert np.all(ln_g == 1.0) and np.all(ln_b == 0.0), \
        "kernel assumes identity LN affine"

    key = (T, B)
    if key not in _NC_CACHE:
        _NC_CACHE[key] = build_nc(T=T, B=B)
    nc = _NC_CACHE[key]

    in_maps = make_in_maps(inputs, T=T, B=B)
    res = run_bass_kernel_spmd(nc, in_maps, list(range(N_CORES)))
    return combine_outputs(res.results, inputs["pad_start_index"],
                           inputs["W_fc"], inputs["b_fc"], T=T, B=B)


if __name__ == "__main__":
    import reference
    inp = reference.setup_inputs()
    out = kernel(**{k: np.asarray(v) for k, v in inp.items()})
    ref = np.asarray(reference.reference(**inp))
    err = np.abs(out - ref).max() / np.abs(ref).max()
    print(f"Relative error: {err:.3e}")
